# revision 10
# baseline (speedup 1.0000x reference)
"""Causal multi-head attention on 8 Trainium2 NeuronCores.

Tensor-parallel over heads: 16 heads -> 2 heads per core (128 of the 1024
model dims per core). Each core computes q/k/v projections for its head
slice, causal attention, and its partial output projection (row-slice of
Wo); the host sums the 8 bf16 partials (+bv@wo... no: bias handled on
device for q/k/v; host adds bo).

All matmuls run bf16 (full PE rate, FWL-eligible 128-col weights).
Layouts (partition dim first):
  xT     [1024, 8192] bf16  x transposed (host-prepared)
  qT/kT  [128, 2048]/b      head dims on partitions (h0: 0-63, h1: 64-127)
  vplus  [128, 16, 256]/b   per key tile: [v_h0(64)|ones|0pad(63)] x2 heads
                            (128-col stationary per head -> FWL; ones row
                            yields the softmax denominator in psum row 64)
  scores ps_s [128, 2, 512] k @ qT per key tile, 2 heads row-packed
  ctx    ps_c [128, 512]    rows 0-63 ctx^T, row 64 denom, 65-127 zeros

Causal structure: for diagonal key tiles (o = kt - 4*qc >= 0) only query
columns >= 128*o are computed (scores/exp/ctx all trimmed); the single
128-wide partial strip is masked with a [128,128] triangular mask.

Emission interleaves batch b+1's projections into batch b's attention at
key-tile granularity, and ctx matmuls LAG behind their scores, so the
in-order PE queue never stalls waiting for the scalar engine's exp.
"""

import numpy as np
from contextlib import ExitStack

import concourse.bass as bass
import concourse.mybir as mybir
import concourse.tile as tile
from concourse import bacc
from concourse import bass_utils
from concourse.masks import make_identity

F32 = mybir.dt.float32
BF16 = mybir.dt.bfloat16
AF = mybir.ActivationFunctionType

B, S, D = 4, 2048, 1024
H, DH = 16, 64
NCORES = 8
DHC = 128           # head dims per core (2 heads x 64)
BS = B * S          # 8192
QC = 512            # q-chunk width
NQC = S // QC       # 4 q-chunks per batch
NKT = S // 128      # 16 key tiles per batch
NKD = D // 128      # 8 contraction tiles for projections
LAG = 2             # ctx matmuls trail scores by this many key-tile units

_CACHE = {}


def _build(dump=False):
    nc = bacc.Bacc("TRN2", target_bir_lowering=False, debug=False)
    xT = nc.dram_tensor("xT", [D, BS], BF16, kind="ExternalInput").ap()
    wqkv = nc.dram_tensor("wqkv", [D, 3 * DHC], BF16, kind="ExternalInput").ap()
    bqkv = nc.dram_tensor("bqkv", [DHC, 3], F32, kind="ExternalInput").ap()
    wo = nc.dram_tensor("wo", [DHC, D], BF16, kind="ExternalInput").ap()
    cmask = nc.dram_tensor("cmask", [128, 128], BF16, kind="ExternalInput").ap()
    out = nc.dram_tensor("out", [BS, D], BF16, kind="ExternalOutput").ap()
    if dump:
        d_qT = nc.dram_tensor("d_qT", [128, S], BF16, kind="ExternalOutput").ap()
        d_kT = nc.dram_tensor("d_kT", [128, S], BF16, kind="ExternalOutput").ap()
        d_vp = nc.dram_tensor("d_vp", [128, NKT, 256], BF16,
                              kind="ExternalOutput").ap()
        d_exp = nc.dram_tensor("d_exp", [4, 128, 2, QC], BF16,
                               kind="ExternalOutput").ap()
        d_den = nc.dram_tensor("d_den", [NQC * 2, QC], F32,
                               kind="ExternalOutput").ap()
        d_ctxT = nc.dram_tensor("d_ctxT", [128, S], BF16,
                                kind="ExternalOutput").ap()

    xT_r = xT.rearrange("(t p) c -> p t c", p=128)

    with tile.TileContext(nc) as tc:
        with ExitStack() as ctx:
            consts = ctx.enter_context(tc.tile_pool(name="consts", bufs=1))
            big = ctx.enter_context(tc.tile_pool(name="big", bufs=2))
            bigc = ctx.enter_context(tc.tile_pool(name="bigc", bufs=2))
            work = ctx.enter_context(tc.tile_pool(name="work", bufs=2))
            expp = ctx.enter_context(tc.tile_pool(name="expp", bufs=5))
            outp = ctx.enter_context(tc.tile_pool(name="outp", bufs=3))
            small = ctx.enter_context(tc.tile_pool(name="small", bufs=4))
            psA = ctx.enter_context(tc.tile_pool(name="psA", bufs=2, space="PSUM"))
            psB = ctx.enter_context(tc.tile_pool(name="psB", bufs=1, space="PSUM"))
            psC = ctx.enter_context(tc.tile_pool(name="psC", bufs=2, space="PSUM"))

            # ---- constants ----
            t_w = consts.tile([128, NKD, 3 * DHC], BF16, tag="w")
            nc.gpsimd.dma_start(t_w, wqkv.rearrange("(t p) c -> p t c", p=128))
            t_bqkv = consts.tile([DHC, 3], F32, tag="bqkv")
            nc.gpsimd.dma_start(t_bqkv, bqkv)
            t_wo = consts.tile([DHC, D], BF16, tag="wo")
            nc.gpsimd.dma_start(t_wo, wo)
            t_mask = consts.tile([128, 128], BF16, tag="mask")
            nc.gpsimd.dma_start(t_mask, cmask)
            t_idf = consts.tile([128, 128], F32, tag="idf")
            make_identity(nc, t_idf)
            t_id = consts.tile([128, 128], BF16, tag="id")
            nc.vector.tensor_copy(t_id, t_idf)

            qT = {}
            kT = {}
            vplus = {}
            ctxT = {}
            ps_ctx = {}

            def proj_stages(b):
                """Emission closures for batch b's projections: per qc a DMA
                stage, 3 matmul+move stages, and a v-transpose stage."""
                xts = {}

                def mk_dma(qc):
                    def f():
                        if qc == 0:
                            qT[b] = big.tile([128, S], BF16, tag="qT",
                                             name=f"qT{b}")
                            kT[b] = big.tile([128, S], BF16, tag="kT",
                                             name=f"kT{b}")
                            vplus[b] = big.tile([128, NKT, 256], BF16,
                                                tag="vplus", name=f"vplus{b}")
                            nc.gpsimd.memset(vplus[b][:, :, 64:65], 1.0)
                            nc.gpsimd.memset(vplus[b][:, :, 192:193], 1.0)
                            if b < 2:  # zero pads once per ring slot
                                nc.gpsimd.memset(vplus[b][:, :, 65:128], 0.0)
                                nc.gpsimd.memset(vplus[b][:, :, 193:256], 0.0)
                        j0 = b * S + qc * QC
                        xt = work.tile([128, NKD, QC], BF16, tag="xt")
                        nc.sync.dma_start(xt, xT_r[:, :, j0:j0 + QC])
                        xts[qc] = xt
                    return f

                def mk_mm(qc, pi):
                    def f():
                        ps = psC.tile([128, QC], F32, tag="pp")
                        for kd in range(NKD):
                            nc.tensor.matmul(
                                ps,
                                t_w[:, kd, pi * DHC:(pi + 1) * DHC],
                                xts[qc][:, kd, :],
                                start=(kd == 0),
                                stop=(kd == NKD - 1),
                            )
                        bias_ap = t_bqkv[:, pi:pi + 1]
                        q0 = qc * QC
                        if pi == 0:
                            nc.vector.tensor_scalar_add(
                                qT[b][:, q0:q0 + QC], ps, bias_ap)
                        elif pi == 1:
                            nc.vector.tensor_scalar_add(
                                kT[b][:, q0:q0 + QC], ps, bias_ap)
                        else:
                            vst = small.tile([128, QC], BF16, tag="vstage")
                            nc.vector.tensor_scalar_add(vst, ps, bias_ap)
                            xts["vst%d" % qc] = vst
                    return f

                def mk_vt(qc):
                    def f():
                        vst = xts["vst%d" % qc]
                        for tt in range(4):
                            loc = qc * 4 + tt
                            ps_t = psC.tile([128, 128], BF16, tag="pp",
                                            name="ps_t")
                            nc.tensor.transpose(
                                ps_t, vst[:, tt * 128:(tt + 1) * 128], t_id)
                            nc.vector.tensor_copy(
                                vplus[b][:, loc, :].rearrange(
                                    "p (h c) -> p h c", c=128)[:, :, 0:64],
                                ps_t.rearrange("p (h c) -> p h c", c=64),
                            )
                    return f

                stages = []
                for qc in range(NQC):
                    stages.append(mk_dma(qc))
                    for pi in range(3):
                        stages.append(mk_mm(qc, pi))
                    stages.append(mk_vt(qc))
                return stages

            exps = {}

            def emit_scores_exp(b, qc, kt):
                q0 = qc * QC
                o = kt - 4 * qc
                tr = 128 * o if o > 0 else 0  # trimmed query prefix
                ps_s = psA.tile([128, 2, QC], F32, tag="scores")
                nc.tensor.matmul(
                    ps_s[:, 0, tr:QC],
                    kT[b][0:64, kt * 128:(kt + 1) * 128],
                    qT[b][0:64, q0 + tr:q0 + QC],
                    start=True, stop=True,
                )
                nc.tensor.matmul(
                    ps_s[:, 1, tr:QC],
                    kT[b][64:128, kt * 128:(kt + 1) * 128],
                    qT[b][64:128, q0 + tr:q0 + QC],
                    start=True, stop=True,
                    tile_position=(64, 0),
                )
                t_exp = expp.tile([128, 2, QC], BF16, tag="exp")
                nc.scalar.activation(
                    t_exp[:, :, tr:QC], ps_s[:, :, tr:QC], AF.Exp, scale=0.125)
                if o >= 0:  # diagonal tile: mask the partial 128-wide strip
                    nc.gpsimd.tensor_mul(
                        t_exp[:, :, tr:tr + 128],
                        t_exp[:, :, tr:tr + 128],
                        t_mask[:, None, :].broadcast_to([128, 2, 128]),
                    )
                if dump and b == 0 and qc == 0:
                    nc.sync.dma_start(d_exp[kt], t_exp)
                exps[(b, qc, kt)] = t_exp

            def emit_ctx(b, qc, kt):
                if kt == 0:
                    ps_ctx[(b, 0)] = psB.tile([128, QC], F32, tag="ctx0", name="ps_c0")
                    ps_ctx[(b, 1)] = psB.tile([128, QC], F32, tag="ctx1", name="ps_c1")
                o = kt - 4 * qc
                tr = 128 * o if o > 0 else 0
                nkt = 4 * qc + 4
                t_exp = exps.pop((b, qc, kt))
                for h in (0, 1):
                    nc.tensor.matmul(
                        ps_ctx[(b, h)][:, tr:QC],
                        vplus[b][:, kt, h * 128:(h + 1) * 128],
                        t_exp[:, h, tr:QC],
                        start=(kt == 0), stop=(kt == nkt - 1),
                    )

            def emit_norm_oproj(b, qc):
                if qc == 0:
                    ctxT[b] = bigc.tile([128, S], BF16, tag="ctxT",
                                        name=f"ctxT{b}")
                q0 = qc * QC
                for h in (0, 1):
                    ps_c = ps_ctx[(b, h)]
                    if dump and b == 0:
                        t_dd = small.tile([1, QC], F32, tag="dden", name="t_dd")
                        nc.vector.tensor_copy(t_dd, ps_c[64:65, :])
                        nc.sync.dma_start(d_den[qc * 2 + h:qc * 2 + h + 1, :], t_dd)
                    t_d = small.tile([1, QC], F32, tag="den", name="t_d")
                    nc.vector.tensor_copy(t_d, ps_c[64:65, :])
                    t_r = small.tile([1, QC], F32, tag="recip")
                    nc.vector.reciprocal_approx_fast(t_r, t_d)
                    t_bc = small.tile([64, QC], F32, tag="bcast")
                    nc.gpsimd.partition_broadcast(t_bc, t_r)
                    nc.vector.tensor_mul(
                        ctxT[b][h * 64:(h + 1) * 64, q0:q0 + QC],
                        ps_c[0:64, :],
                        t_bc,
                    )
                for qi in range(QC // 128):
                    qt = qc * 4 + qi
                    r0 = b * S + qt * 128
                    t_o = outp.tile([128, D], BF16, tag="out")
                    for ch in range(2):
                        ps_o = psC.tile([128, QC], F32, tag="pp", name="ps_o")
                        nc.tensor.matmul(
                            ps_o,
                            ctxT[b][:, qt * 128:(qt + 1) * 128],
                            t_wo[:, ch * QC:(ch + 1) * QC],
                            start=True, stop=True,
                        )
                        dst = t_o[:, ch * QC:(ch + 1) * QC]
                        if qi == 0 and ch == 0:
                            nc.scalar.copy(dst, ps_o)
                        else:
                            nc.vector.tensor_copy(dst, ps_o)
                    nc.sync.dma_start(out[r0:r0 + 128, :], t_o)

            def emit_batch(b):
                """Attention for batch b, interleaved with batch b+1's
                projection stages; ctx matmuls trail scores by LAG units."""
                units = [(qc, kt) for qc in range(NQC)
                         for kt in range(4 * qc + 4)]
                stages = proj_stages(b + 1) if b + 1 < B else []
                nu = len(units) + LAG
                si = 0
                for i in range(nu):
                    if i < len(units):
                        emit_scores_exp(b, *units[i])
                    j = i - LAG
                    if j >= 0:
                        qc, kt = units[j]
                        emit_ctx(b, qc, kt)
                        if kt == 4 * qc + 3:
                            emit_norm_oproj(b, qc)
                    want = (len(stages) * (i + 1)) // nu
                    while si < want:
                        stages[si]()
                        si += 1

            for st in proj_stages(0):
                st()
            if dump:
                nc.sync.dma_start(d_qT, qT[0])
                nc.sync.dma_start(d_kT, kT[0])
                nc.sync.dma_start(d_vp, vplus[0])
            for b in range(B):
                emit_batch(b)
                if dump and b == 0:
                    nc.sync.dma_start(d_ctxT, ctxT[0])

    nc.compile()
    return nc


def _host_inputs(x, wq, bq, wk, bk, wv, bv, wo, bo):
    import ml_dtypes
    bf16 = ml_dtypes.bfloat16
    x = np.asarray(x, dtype=np.float32).reshape(BS, D)
    xT = np.ascontiguousarray(x.T.astype(bf16))
    p = np.arange(128)[:, None]
    j = np.arange(128)[None, :]
    cmask = (j >= p).astype(bf16)
    wq, wk, wv, wo = (np.asarray(a, dtype=np.float32) for a in (wq, wk, wv, wo))
    bq, bk, bv, bo = (np.asarray(a, dtype=np.float32) for a in (bq, bk, bv, bo))
    in_maps = []
    for c in range(NCORES):
        sl = slice(c * DHC, (c + 1) * DHC)
        wqkv = np.ascontiguousarray(
            np.concatenate([wq[:, sl], wk[:, sl], wv[:, sl]], axis=1).astype(bf16)
        )
        bqkv = np.ascontiguousarray(np.stack([bq[sl], bk[sl], bv[sl]], axis=1))
        in_maps.append({
            "xT": xT,
            "wqkv": wqkv,
            "bqkv": bqkv,
            "wo": np.ascontiguousarray(wo[sl, :].astype(bf16)),
            "cmask": cmask,
        })
    return in_maps


def kernel(x, wq, bq, wk, bk, wv, bv, wo, bo, _trace=False, _tmpdir=None):
    if "nc" not in _CACHE:
        _CACHE["nc"] = _build()
    nc = _CACHE["nc"]
    in_maps = _host_inputs(x, wq, bq, wk, bk, wv, bv, wo, bo)
    res = bass_utils.run_bass_kernel_spmd(
        nc, in_maps, core_ids=list(range(NCORES)), trace=_trace, tmpdir=_tmpdir
    )
    _CACHE["last_results"] = res
    acc = np.zeros((BS, D), dtype=np.float32)
    for c in range(NCORES):
        acc += res.results[c]["out"].astype(np.float32)
    acc += np.asarray(bo, dtype=np.float32)[None, :]
    return acc.reshape(B, S, D)


# revision 11
# speedup vs baseline: 1.5068x; 1.5068x over previous
"""Causal multi-head attention on 8 Trainium2 NeuronCores.

Tensor-parallel over heads: 16 heads -> 2 heads per core (128 of the 1024
model dims per core). Each core computes q/k/v projections for its head
slice, causal attention, and its partial output projection (row-slice of
Wo); the host sums the 8 bf16 partials (+bv@wo... no: bias handled on
device for q/k/v; host adds bo).

All matmuls run bf16 (full PE rate, FWL-eligible 128-col weights).
Layouts (partition dim first):
  xT     [1024, 8192] bf16  x transposed (host-prepared)
  qT/kT  [128, 2048]/b      head dims on partitions (h0: 0-63, h1: 64-127)
  vplus  [128, 16, 256]/b   per key tile: [v_h0(64)|ones|0pad(63)] x2 heads
                            (128-col stationary per head -> FWL; ones row
                            yields the softmax denominator in psum row 64)
  scores ps_s [128, 2, 512] k @ qT per key tile, 2 heads row-packed
  ctx    ps_c [128, 512]    rows 0-63 ctx^T, row 64 denom, 65-127 zeros

Causal structure: for diagonal key tiles (o = kt - 4*qc >= 0) only query
columns >= 128*o are computed (scores/exp/ctx all trimmed); the single
128-wide partial strip is masked with a [128,128] triangular mask.

Emission interleaves batch b+1's projections into batch b's attention at
key-tile granularity, and ctx matmuls LAG behind their scores, so the
in-order PE queue never stalls waiting for the scalar engine's exp.
"""

import numpy as np
from contextlib import ExitStack

import concourse.bass as bass
import concourse.mybir as mybir
import concourse.tile as tile
from concourse import bacc
from concourse import bass_utils
from concourse.masks import make_identity

F32 = mybir.dt.float32
BF16 = mybir.dt.bfloat16
AF = mybir.ActivationFunctionType

B, S, D = 4, 2048, 1024
H, DH = 16, 64
NCORES = 8
DHC = 128           # head dims per core (2 heads x 64)
BS = B * S          # 8192
QC = 512            # q-chunk width
NQC = S // QC       # 4 q-chunks per batch
NKT = S // 128      # 16 key tiles per batch
NKD = D // 128      # 8 contraction tiles for projections
LAG = 2             # ctx matmuls trail scores by this many key-tile units

_CACHE = {}


def _build(dump=False):
    nc = bacc.Bacc("TRN2", target_bir_lowering=False, debug=False)
    xT = nc.dram_tensor("xT", [D, BS], BF16, kind="ExternalInput").ap()
    wqkv = nc.dram_tensor("wqkv", [D, 3 * DHC], BF16, kind="ExternalInput").ap()
    bqkv = nc.dram_tensor("bqkv", [DHC, 3], F32, kind="ExternalInput").ap()
    wo = nc.dram_tensor("wo", [DHC, D], BF16, kind="ExternalInput").ap()
    cmask = nc.dram_tensor("cmask", [128, 128], BF16, kind="ExternalInput").ap()
    out = nc.dram_tensor("out", [BS, D], BF16, kind="ExternalOutput").ap()
    if dump:
        d_qT = nc.dram_tensor("d_qT", [128, S], BF16, kind="ExternalOutput").ap()
        d_kT = nc.dram_tensor("d_kT", [128, S], BF16, kind="ExternalOutput").ap()
        d_vp = nc.dram_tensor("d_vp", [128, NKT, 256], BF16,
                              kind="ExternalOutput").ap()
        d_exp = nc.dram_tensor("d_exp", [4, 128, 2, QC], BF16,
                               kind="ExternalOutput").ap()
        d_den = nc.dram_tensor("d_den", [NQC * 2, QC], F32,
                               kind="ExternalOutput").ap()
        d_ctxT = nc.dram_tensor("d_ctxT", [128, S], BF16,
                                kind="ExternalOutput").ap()

    xT_r = xT.rearrange("(t p) c -> p t c", p=128)

    with tile.TileContext(nc) as tc:
        with ExitStack() as ctx:
            consts = ctx.enter_context(tc.tile_pool(name="consts", bufs=1))
            big = ctx.enter_context(tc.tile_pool(name="big", bufs=2))
            bigc = ctx.enter_context(tc.tile_pool(name="bigc", bufs=2))
            work = ctx.enter_context(tc.tile_pool(name="work", bufs=2))
            expp = ctx.enter_context(tc.tile_pool(name="expp", bufs=5))
            outp = ctx.enter_context(tc.tile_pool(name="outp", bufs=3))
            small = ctx.enter_context(tc.tile_pool(name="small", bufs=4))
            psA = ctx.enter_context(tc.tile_pool(name="psA", bufs=2, space="PSUM"))
            psB = ctx.enter_context(tc.tile_pool(name="psB", bufs=1, space="PSUM"))
            psC = ctx.enter_context(tc.tile_pool(name="psC", bufs=2, space="PSUM"))

            # ---- constants ----
            t_w = consts.tile([128, NKD, 3 * DHC], BF16, tag="w")
            nc.gpsimd.dma_start(t_w, wqkv.rearrange("(t p) c -> p t c", p=128))
            t_bqkv = consts.tile([DHC, 3], F32, tag="bqkv")
            nc.gpsimd.dma_start(t_bqkv, bqkv)
            t_wo = consts.tile([DHC, D], BF16, tag="wo")
            nc.gpsimd.dma_start(t_wo, wo)
            t_mask = consts.tile([128, 128], BF16, tag="mask")
            nc.gpsimd.dma_start(t_mask, cmask)
            t_idf = consts.tile([128, 128], F32, tag="idf")
            make_identity(nc, t_idf)
            t_id = consts.tile([128, 128], BF16, tag="id")
            nc.vector.tensor_copy(t_id, t_idf)

            qT = {}
            kT = {}
            vplus = {}
            ctxT = {}
            ps_ctx = {}

            def proj_stages(b):
                """Emission closures for batch b's projections: per qc a DMA
                stage, 3 matmul+move stages, and a v-transpose stage."""
                xts = {}

                def mk_dma(qc):
                    def f():
                        if qc == 0:
                            qT[b] = big.tile([128, S], BF16, tag="qT",
                                             name=f"qT{b}")
                            kT[b] = big.tile([128, S], BF16, tag="kT",
                                             name=f"kT{b}")
                            vplus[b] = big.tile([128, NKT, 256], BF16,
                                                tag="vplus", name=f"vplus{b}")
                            nc.gpsimd.memset(vplus[b][:, :, 64:65], 1.0)
                            nc.gpsimd.memset(vplus[b][:, :, 192:193], 1.0)
                            if b < 2:  # zero pads once per ring slot
                                nc.gpsimd.memset(vplus[b][:, :, 65:128], 0.0)
                                nc.gpsimd.memset(vplus[b][:, :, 193:256], 0.0)
                        j0 = b * S + qc * QC
                        xt = work.tile([128, NKD, QC], BF16, tag="xt")
                        nc.sync.dma_start(xt, xT_r[:, :, j0:j0 + QC])
                        xts[qc] = xt
                    return f

                def mk_mm(qc, pi):
                    def f():
                        ps = psC.tile([128, QC], F32, tag="pp")
                        for kd in range(NKD):
                            nc.tensor.matmul(
                                ps,
                                t_w[:, kd, pi * DHC:(pi + 1) * DHC],
                                xts[qc][:, kd, :],
                                start=(kd == 0),
                                stop=(kd == NKD - 1),
                            )
                        bias_ap = t_bqkv[:, pi:pi + 1]
                        q0 = qc * QC
                        if pi == 0:
                            nc.vector.tensor_scalar_add(
                                qT[b][:, q0:q0 + QC], ps, bias_ap)
                        elif pi == 1:
                            nc.vector.tensor_scalar_add(
                                kT[b][:, q0:q0 + QC], ps, bias_ap)
                        else:
                            vst = small.tile([128, QC], BF16, tag="vstage")
                            nc.vector.tensor_scalar_add(vst, ps, bias_ap)
                            xts["vst%d" % qc] = vst
                    return f

                def mk_vt(qc):
                    def f():
                        vst = xts["vst%d" % qc]
                        for tt in range(4):
                            loc = qc * 4 + tt
                            ps_t = psC.tile([128, 128], BF16, tag="pp",
                                            name="ps_t")
                            nc.tensor.transpose(
                                ps_t, vst[:, tt * 128:(tt + 1) * 128], t_id)
                            nc.vector.tensor_copy(
                                vplus[b][:, loc, :].rearrange(
                                    "p (h c) -> p h c", c=128)[:, :, 0:64],
                                ps_t.rearrange("p (h c) -> p h c", c=64),
                            )
                    return f

                stages = []
                for qc in range(NQC):
                    stages.append(mk_dma(qc))
                    for pi in range(3):
                        stages.append(mk_mm(qc, pi))
                    stages.append(mk_vt(qc))
                return stages

            exps = {}

            def emit_scores_exp(b, qc, kt):
                q0 = qc * QC
                o = kt - 4 * qc
                tr = 128 * o if o > 0 else 0  # trimmed query prefix
                ps_s = psA.tile([128, 2, QC], F32, tag="scores")
                nc.tensor.matmul(
                    ps_s[:, 0, tr:QC],
                    kT[b][0:64, kt * 128:(kt + 1) * 128],
                    qT[b][0:64, q0 + tr:q0 + QC],
                    start=True, stop=True,
                )
                nc.tensor.matmul(
                    ps_s[:, 1, tr:QC],
                    kT[b][64:128, kt * 128:(kt + 1) * 128],
                    qT[b][64:128, q0 + tr:q0 + QC],
                    start=True, stop=True,
                    tile_position=(64, 0),
                )
                t_exp = expp.tile([128, 2, QC], BF16, tag="exp")
                nc.scalar.activation(
                    t_exp[:, :, tr:QC], ps_s[:, :, tr:QC], AF.Exp, scale=0.125)
                if o >= 0:  # diagonal tile: mask the partial 128-wide strip
                    nc.vector.tensor_mul(
                        t_exp[:, :, tr:tr + 128],
                        t_exp[:, :, tr:tr + 128],
                        t_mask[:, None, :].broadcast_to([128, 2, 128]),
                    )
                if dump and b == 0 and qc == 0:
                    nc.sync.dma_start(d_exp[kt], t_exp)
                exps[(b, qc, kt)] = t_exp

            def emit_ctx(b, qc, kt):
                if kt == 0:
                    ps_ctx[(b, 0)] = psB.tile([128, QC], F32, tag="ctx0", name="ps_c0")
                    ps_ctx[(b, 1)] = psB.tile([128, QC], F32, tag="ctx1", name="ps_c1")
                o = kt - 4 * qc
                tr = 128 * o if o > 0 else 0
                nkt = 4 * qc + 4
                t_exp = exps.pop((b, qc, kt))
                for h in (0, 1):
                    nc.tensor.matmul(
                        ps_ctx[(b, h)][:, tr:QC],
                        vplus[b][:, kt, h * 128:(h + 1) * 128],
                        t_exp[:, h, tr:QC],
                        start=(kt == 0), stop=(kt == nkt - 1),
                    )

            def emit_norm_oproj(b, qc):
                if qc == 0:
                    ctxT[b] = bigc.tile([128, S], BF16, tag="ctxT",
                                        name=f"ctxT{b}")
                q0 = qc * QC
                for h in (0, 1):
                    ps_c = ps_ctx[(b, h)]
                    if dump and b == 0:
                        t_dd = small.tile([1, QC], F32, tag="dden", name="t_dd")
                        nc.vector.tensor_copy(t_dd, ps_c[64:65, :])
                        nc.sync.dma_start(d_den[qc * 2 + h:qc * 2 + h + 1, :], t_dd)
                    t_d = small.tile([1, QC], F32, tag="den", name="t_d")
                    nc.vector.tensor_copy(t_d, ps_c[64:65, :])
                    t_r = small.tile([1, QC], F32, tag="recip")
                    nc.vector.reciprocal_approx_fast(t_r, t_d)
                    t_bc = small.tile([64, QC], F32, tag="bcast")
                    nc.gpsimd.partition_broadcast(t_bc, t_r)
                    nc.vector.tensor_mul(
                        ctxT[b][h * 64:(h + 1) * 64, q0:q0 + QC],
                        ps_c[0:64, :],
                        t_bc,
                    )
                for qi in range(QC // 128):
                    qt = qc * 4 + qi
                    r0 = b * S + qt * 128
                    t_o = outp.tile([128, D], BF16, tag="out")
                    for ch in range(2):
                        ps_o = psC.tile([128, QC], F32, tag="pp", name="ps_o")
                        nc.tensor.matmul(
                            ps_o,
                            ctxT[b][:, qt * 128:(qt + 1) * 128],
                            t_wo[:, ch * QC:(ch + 1) * QC],
                            start=True, stop=True,
                        )
                        dst = t_o[:, ch * QC:(ch + 1) * QC]
                        if qi == 0 and ch == 0:
                            nc.scalar.copy(dst, ps_o)
                        else:
                            nc.vector.tensor_copy(dst, ps_o)
                    nc.sync.dma_start(out[r0:r0 + 128, :], t_o)

            def emit_batch(b):
                """Attention for batch b, interleaved with batch b+1's
                projection stages; ctx matmuls trail scores by LAG units."""
                units = [(qc, kt) for qc in range(NQC)
                         for kt in range(4 * qc + 4)]
                stages = proj_stages(b + 1) if b + 1 < B else []
                nu = len(units) + LAG
                si = 0
                for i in range(nu):
                    if i < len(units):
                        emit_scores_exp(b, *units[i])
                    j = i - LAG
                    if j >= 0:
                        qc, kt = units[j]
                        emit_ctx(b, qc, kt)
                        if kt == 4 * qc + 3:
                            emit_norm_oproj(b, qc)
                    want = (len(stages) * (i + 1)) // nu
                    while si < want:
                        stages[si]()
                        si += 1

            for st in proj_stages(0):
                st()
            if dump:
                nc.sync.dma_start(d_qT, qT[0])
                nc.sync.dma_start(d_kT, kT[0])
                nc.sync.dma_start(d_vp, vplus[0])
            for b in range(B):
                emit_batch(b)
                if dump and b == 0:
                    nc.sync.dma_start(d_ctxT, ctxT[0])

    nc.compile()
    return nc


def _host_inputs(x, wq, bq, wk, bk, wv, bv, wo, bo):
    import ml_dtypes
    bf16 = ml_dtypes.bfloat16
    x = np.asarray(x, dtype=np.float32).reshape(BS, D)
    xT = np.ascontiguousarray(x.T.astype(bf16))
    p = np.arange(128)[:, None]
    j = np.arange(128)[None, :]
    cmask = (j >= p).astype(bf16)
    wq, wk, wv, wo = (np.asarray(a, dtype=np.float32) for a in (wq, wk, wv, wo))
    bq, bk, bv, bo = (np.asarray(a, dtype=np.float32) for a in (bq, bk, bv, bo))
    in_maps = []
    for c in range(NCORES):
        sl = slice(c * DHC, (c + 1) * DHC)
        wqkv = np.ascontiguousarray(
            np.concatenate([wq[:, sl], wk[:, sl], wv[:, sl]], axis=1).astype(bf16)
        )
        bqkv = np.ascontiguousarray(np.stack([bq[sl], bk[sl], bv[sl]], axis=1))
        in_maps.append({
            "xT": xT,
            "wqkv": wqkv,
            "bqkv": bqkv,
            "wo": np.ascontiguousarray(wo[sl, :].astype(bf16)),
            "cmask": cmask,
        })
    return in_maps


def kernel(x, wq, bq, wk, bk, wv, bv, wo, bo, _trace=False, _tmpdir=None):
    if "nc" not in _CACHE:
        _CACHE["nc"] = _build()
    nc = _CACHE["nc"]
    in_maps = _host_inputs(x, wq, bq, wk, bk, wv, bv, wo, bo)
    res = bass_utils.run_bass_kernel_spmd(
        nc, in_maps, core_ids=list(range(NCORES)), trace=_trace, tmpdir=_tmpdir
    )
    _CACHE["last_results"] = res
    acc = np.zeros((BS, D), dtype=np.float32)
    for c in range(NCORES):
        acc += res.results[c]["out"].astype(np.float32)
    acc += np.asarray(bo, dtype=np.float32)[None, :]
    return acc.reshape(B, S, D)


# revision 16
# speedup vs baseline: 1.5699x; 1.0419x over previous
"""Causal multi-head attention on 8 Trainium2 NeuronCores.

Tensor-parallel over heads: 16 heads -> 2 heads per core (128 of the 1024
model dims per core). Each core computes q/k/v projections for its head
slice, causal attention, and its partial output projection (row-slice of
Wo); the host sums the 8 bf16 partials (+bv@wo... no: bias handled on
device for q/k/v; host adds bo).

All matmuls run bf16 (full PE rate, FWL-eligible 128-col weights).
Layouts (partition dim first):
  xT     [1024, 8192] bf16  x transposed (host-prepared)
  qT/kT  [128, 2048]/b      head dims on partitions (h0: 0-63, h1: 64-127)
  vplus  [128, 16, 256]/b   per key tile: [v_h0(64)|ones|0pad(63)] x2 heads
                            (128-col stationary per head -> FWL; ones row
                            yields the softmax denominator in psum row 64)
  scores ps_s [128, 2, 512] k @ qT per key tile, 2 heads row-packed
  ctx    ps_c [128, 512]    rows 0-63 ctx^T, row 64 denom, 65-127 zeros

Causal structure: for diagonal key tiles (o = kt - 4*qc >= 0) only query
columns >= 128*o are computed (scores/exp/ctx all trimmed); the single
128-wide partial strip is masked with a [128,128] triangular mask.

Emission interleaves batch b+1's projections into batch b's attention at
key-tile granularity, and ctx matmuls LAG behind their scores, so the
in-order PE queue never stalls waiting for the scalar engine's exp.
"""

import numpy as np
from contextlib import ExitStack

import concourse.bass as bass
import concourse.mybir as mybir
import concourse.tile as tile
from concourse import bacc
from concourse import bass_utils
from concourse.masks import make_identity

F32 = mybir.dt.float32
BF16 = mybir.dt.bfloat16
AF = mybir.ActivationFunctionType

B, S, D = 4, 2048, 1024
H, DH = 16, 64
NCORES = 8
DHC = 128           # head dims per core (2 heads x 64)
BS = B * S          # 8192
QC = 512            # q-chunk width
NQC = S // QC       # 4 q-chunks per batch
NKT = S // 128      # 16 key tiles per batch
NKD = D // 128      # 8 contraction tiles for projections
LAG = 2             # ctx matmuls trail scores by this many key-tile units

_CACHE = {}


def _build(dump=False):
    nc = bacc.Bacc("TRN2", target_bir_lowering=False, debug=False)
    xT = nc.dram_tensor("xT", [D, BS], BF16, kind="ExternalInput").ap()
    wqkv = nc.dram_tensor("wqkv", [D, 3 * DHC], BF16, kind="ExternalInput").ap()
    bqkv = nc.dram_tensor("bqkv", [DHC, 3], F32, kind="ExternalInput").ap()
    wo = nc.dram_tensor("wo", [DHC, D], BF16, kind="ExternalInput").ap()
    cmask = nc.dram_tensor("cmask", [128, 128], BF16, kind="ExternalInput").ap()
    out = nc.dram_tensor("out", [BS, D], BF16, kind="ExternalOutput").ap()
    if dump:
        d_qT = nc.dram_tensor("d_qT", [128, S], BF16, kind="ExternalOutput").ap()
        d_kT = nc.dram_tensor("d_kT", [128, S], BF16, kind="ExternalOutput").ap()
        d_vp = nc.dram_tensor("d_vp", [128, NKT, 256], BF16,
                              kind="ExternalOutput").ap()
        d_exp = nc.dram_tensor("d_exp", [4, 128, 2, QC], BF16,
                               kind="ExternalOutput").ap()
        d_den = nc.dram_tensor("d_den", [NQC * 2, QC], F32,
                               kind="ExternalOutput").ap()
        d_ctxT = nc.dram_tensor("d_ctxT", [128, S], BF16,
                                kind="ExternalOutput").ap()

    xT_r = xT.rearrange("(t p) c -> p t c", p=128)

    with tile.TileContext(nc) as tc:
        with ExitStack() as ctx:
            consts = ctx.enter_context(tc.tile_pool(name="consts", bufs=1))
            big = ctx.enter_context(tc.tile_pool(name="big", bufs=2))
            bigc = ctx.enter_context(tc.tile_pool(name="bigc", bufs=2))
            work = ctx.enter_context(tc.tile_pool(name="work", bufs=2))
            expp = ctx.enter_context(tc.tile_pool(name="expp", bufs=5))
            outp = ctx.enter_context(tc.tile_pool(name="outp", bufs=3))
            small = ctx.enter_context(tc.tile_pool(name="small", bufs=4))
            psA = ctx.enter_context(tc.tile_pool(name="psA", bufs=2, space="PSUM"))
            psB = ctx.enter_context(tc.tile_pool(name="psB", bufs=1, space="PSUM"))
            psC = ctx.enter_context(tc.tile_pool(name="psC", bufs=2, space="PSUM"))

            # ---- constants ----
            wqkv_r = wqkv.rearrange("(t p) c -> p t c", p=128)
            t_w = consts.tile([128, NKD, 3 * DHC], BF16, tag="w")
            for kd in range(NKD):  # split across DMA queues
                nc.sync.dma_start(t_w[:, kd, :], wqkv_r[:, kd, :])
            t_bqkv = consts.tile([DHC, 3], F32, tag="bqkv")
            nc.gpsimd.dma_start(t_bqkv, bqkv)
            t_wo = consts.tile([DHC, D], BF16, tag="wo")
            nc.gpsimd.dma_start(t_wo, wo)
            t_mask = consts.tile([128, 128], BF16, tag="mask")
            nc.gpsimd.dma_start(t_mask, cmask)
            t_idf = consts.tile([128, 128], F32, tag="idf")
            make_identity(nc, t_idf)
            t_id = consts.tile([128, 128], BF16, tag="id")
            nc.vector.tensor_copy(t_id, t_idf)
            # PE warmup: trip the HAM clock gate to 8/8 while input DMAs land
            ps_warm = psC.tile([128, 128], F32, tag="pp", name="ps_warm")
            for _ in range(100):
                nc.tensor.matmul(ps_warm, t_id, t_id, start=True, stop=True)

            qT = {}
            kT = {}
            vplus = {}
            ctxT = {}
            ps_ctx = {}

            def proj_stages(b):
                """Emission closures for batch b's projections: per qc a DMA
                stage, 3 matmul+move stages, and a v-transpose stage."""
                xts = {}

                def mk_dma(qc):
                    def f():
                        if qc == 0:
                            qT[b] = big.tile([128, S], BF16, tag="qT",
                                             name=f"qT{b}")
                            kT[b] = big.tile([128, S], BF16, tag="kT",
                                             name=f"kT{b}")
                            vplus[b] = big.tile([128, NKT, 256], BF16,
                                                tag="vplus", name=f"vplus{b}")
                            nc.gpsimd.memset(vplus[b][:, :, 64:65], 1.0)
                            nc.gpsimd.memset(vplus[b][:, :, 192:193], 1.0)
                            if b < 2:  # zero pads once per ring slot
                                nc.gpsimd.memset(vplus[b][:, :, 65:128], 0.0)
                                nc.gpsimd.memset(vplus[b][:, :, 193:256], 0.0)
                        j0 = b * S + qc * QC
                        xt = work.tile([128, NKD, QC], BF16, tag="xt")
                        nc.sync.dma_start(xt, xT_r[:, :, j0:j0 + QC])
                        xts[qc] = xt
                    return f

                def mk_mm(qc, pi):
                    def f():
                        ps = psC.tile([128, QC], F32, tag="pp")
                        for kd in range(NKD):
                            nc.tensor.matmul(
                                ps,
                                t_w[:, kd, pi * DHC:(pi + 1) * DHC],
                                xts[qc][:, kd, :],
                                start=(kd == 0),
                                stop=(kd == NKD - 1),
                            )
                        bias_ap = t_bqkv[:, pi:pi + 1]
                        q0 = qc * QC
                        if pi == 0:
                            nc.vector.tensor_scalar_add(
                                qT[b][:, q0:q0 + QC], ps, bias_ap)
                        elif pi == 1:
                            nc.vector.tensor_scalar_add(
                                kT[b][:, q0:q0 + QC], ps, bias_ap)
                        else:
                            vst = small.tile([128, QC], BF16, tag="vstage")
                            nc.vector.tensor_scalar_add(vst, ps, bias_ap)
                            xts["vst%d" % qc] = vst
                    return f

                def mk_vt(qc):
                    def f():
                        vst = xts["vst%d" % qc]
                        for tt in range(4):
                            loc = qc * 4 + tt
                            ps_t = psC.tile([128, 128], BF16, tag="pp",
                                            name="ps_t")
                            nc.tensor.transpose(
                                ps_t, vst[:, tt * 128:(tt + 1) * 128], t_id)
                            nc.vector.tensor_copy(
                                vplus[b][:, loc, :].rearrange(
                                    "p (h c) -> p h c", c=128)[:, :, 0:64],
                                ps_t.rearrange("p (h c) -> p h c", c=64),
                            )
                    return f

                stages = []
                for qc in range(NQC):
                    stages.append(mk_dma(qc))
                    for pi in range(3):
                        stages.append(mk_mm(qc, pi))
                    stages.append(mk_vt(qc))
                return stages

            exps = {}

            def emit_scores_exp(b, qc, kt):
                q0 = qc * QC
                o = kt - 4 * qc
                tr = 128 * o if o > 0 else 0  # trimmed query prefix
                ps_s = psA.tile([128, 2, QC], F32, tag="scores")
                nc.tensor.matmul(
                    ps_s[:, 0, tr:QC],
                    kT[b][0:64, kt * 128:(kt + 1) * 128],
                    qT[b][0:64, q0 + tr:q0 + QC],
                    start=True, stop=True,
                )
                nc.tensor.matmul(
                    ps_s[:, 1, tr:QC],
                    kT[b][64:128, kt * 128:(kt + 1) * 128],
                    qT[b][64:128, q0 + tr:q0 + QC],
                    start=True, stop=True,
                    tile_position=(64, 0),
                )
                t_exp = expp.tile([128, 2, QC], BF16, tag="exp")
                nc.scalar.activation(
                    t_exp[:, :, tr:QC], ps_s[:, :, tr:QC], AF.Exp, scale=0.125)
                if o >= 0:  # diagonal tile: mask the partial 128-wide strip
                    nc.vector.tensor_mul(
                        t_exp[:, :, tr:tr + 128],
                        t_exp[:, :, tr:tr + 128],
                        t_mask[:, None, :].broadcast_to([128, 2, 128]),
                    )
                if dump and b == 0 and qc == 0:
                    nc.sync.dma_start(d_exp[kt], t_exp)
                exps[(b, qc, kt)] = t_exp

            def emit_ctx(b, qc, kt):
                if kt == 0:
                    ps_ctx[(b, 0)] = psB.tile([128, QC], F32, tag="ctx0", name="ps_c0")
                    ps_ctx[(b, 1)] = psB.tile([128, QC], F32, tag="ctx1", name="ps_c1")
                o = kt - 4 * qc
                tr = 128 * o if o > 0 else 0
                nkt = 4 * qc + 4
                t_exp = exps.pop((b, qc, kt))
                for h in (0, 1):
                    nc.tensor.matmul(
                        ps_ctx[(b, h)][:, tr:QC],
                        vplus[b][:, kt, h * 128:(h + 1) * 128],
                        t_exp[:, h, tr:QC],
                        start=(kt == 0), stop=(kt == nkt - 1),
                    )

            def emit_norm_oproj(b, qc):
                if qc == 0:
                    ctxT[b] = bigc.tile([128, S], BF16, tag="ctxT",
                                        name=f"ctxT{b}")
                q0 = qc * QC
                for h in (0, 1):
                    ps_c = ps_ctx[(b, h)]
                    t_d = small.tile([1, QC], F32, tag="den", name="t_d")
                    nc.vector.tensor_copy(t_d, ps_c[64:65, :])
                    if dump and b == 0:
                        nc.sync.dma_start(
                            d_den[qc * 2 + h:qc * 2 + h + 1, :], t_d)
                    t_r = small.tile([1, QC], F32, tag="recip")
                    nc.vector.reciprocal_approx_fast(t_r, t_d)
                    t_bc = small.tile([64, QC], F32, tag="bcast")
                    nc.gpsimd.partition_broadcast(t_bc, t_r)
                    nc.vector.tensor_mul(
                        ctxT[b][h * 64:(h + 1) * 64, q0:q0 + QC],
                        ps_c[0:64, :],
                        t_bc,
                    )
                for qi in range(QC // 128):
                    qt = qc * 4 + qi
                    r0 = b * S + qt * 128
                    t_o = outp.tile([128, D], BF16, tag="out")
                    for ch in range(2):
                        ps_o = psC.tile([128, QC], F32, tag="pp", name="ps_o")
                        nc.tensor.matmul(
                            ps_o,
                            ctxT[b][:, qt * 128:(qt + 1) * 128],
                            t_wo[:, ch * QC:(ch + 1) * QC],
                            start=True, stop=True,
                        )
                        dst = t_o[:, ch * QC:(ch + 1) * QC]
                        if ch == 0 and qi % 2 == 0:
                            nc.scalar.copy(dst, ps_o)
                        else:
                            nc.vector.tensor_copy(dst, ps_o)
                    nc.sync.dma_start(out[r0:r0 + 128, :], t_o)

            def emit_batch(b):
                """Attention for batch b, interleaved with batch b+1's
                projection stages; ctx matmuls trail scores by LAG units."""
                units = [(qc, kt) for qc in range(NQC)
                         for kt in range(4 * qc + 4)]
                stages = proj_stages(b + 1) if b + 1 < B else []
                nu = len(units) + LAG
                si = 0
                for i in range(nu):
                    if i < len(units):
                        emit_scores_exp(b, *units[i])
                    j = i - LAG
                    if j >= 0:
                        qc, kt = units[j]
                        emit_ctx(b, qc, kt)
                        if kt == 4 * qc + 3:
                            emit_norm_oproj(b, qc)
                    want = (len(stages) * (i + 1)) // nu
                    while si < want:
                        stages[si]()
                        si += 1

            for st in proj_stages(0):
                st()
            if dump:
                nc.sync.dma_start(d_qT, qT[0])
                nc.sync.dma_start(d_kT, kT[0])
                nc.sync.dma_start(d_vp, vplus[0])
            for b in range(B):
                emit_batch(b)
                if dump and b == 0:
                    nc.sync.dma_start(d_ctxT, ctxT[0])

    nc.compile()
    return nc


def _host_inputs(x, wq, bq, wk, bk, wv, bv, wo, bo):
    import ml_dtypes
    bf16 = ml_dtypes.bfloat16
    x = np.asarray(x, dtype=np.float32).reshape(BS, D)
    xT = np.ascontiguousarray(x.T.astype(bf16))
    p = np.arange(128)[:, None]
    j = np.arange(128)[None, :]
    cmask = (j >= p).astype(bf16)
    wq, wk, wv, wo = (np.asarray(a, dtype=np.float32) for a in (wq, wk, wv, wo))
    bq, bk, bv, bo = (np.asarray(a, dtype=np.float32) for a in (bq, bk, bv, bo))
    in_maps = []
    for c in range(NCORES):
        sl = slice(c * DHC, (c + 1) * DHC)
        wqkv = np.ascontiguousarray(
            np.concatenate([wq[:, sl], wk[:, sl], wv[:, sl]], axis=1).astype(bf16)
        )
        bqkv = np.ascontiguousarray(np.stack([bq[sl], bk[sl], bv[sl]], axis=1))
        in_maps.append({
            "xT": xT,
            "wqkv": wqkv,
            "bqkv": bqkv,
            "wo": np.ascontiguousarray(wo[sl, :].astype(bf16)),
            "cmask": cmask,
        })
    return in_maps


def kernel(x, wq, bq, wk, bk, wv, bv, wo, bo, _trace=False, _tmpdir=None):
    if "nc" not in _CACHE:
        _CACHE["nc"] = _build()
    nc = _CACHE["nc"]
    in_maps = _host_inputs(x, wq, bq, wk, bk, wv, bv, wo, bo)
    res = bass_utils.run_bass_kernel_spmd(
        nc, in_maps, core_ids=list(range(NCORES)), trace=_trace, tmpdir=_tmpdir
    )
    _CACHE["last_results"] = res
    acc = np.zeros((BS, D), dtype=np.float32)
    for c in range(NCORES):
        acc += res.results[c]["out"].astype(np.float32)
    acc += np.asarray(bo, dtype=np.float32)[None, :]
    return acc.reshape(B, S, D)
